# revision 2
# baseline (speedup 1.0000x reference)
"""KnnLoss Trainium2 kernel.

Problem: B=2, N=8192 points in [0,1)^3, mask (B,N,16). For each point, find
its 8 nearest neighbors (squared L2, via s = 2*q.c - |c|^2 which is a
per-row constant shift of -d2), replace out-of-radius neighbors with the
nearest (self) index, gather mask rows at the neighbor indices, and
accumulate sum_s |mask[n,s] - mask[nn,s]|. Final loss = total / (B*N*k).

Sharding: 8 cores, each handles one (batch, query-block) pair: core c ->
batch c//4, queries (c%4)*2048 .. +2048. Candidates/gather table are the
full per-batch pc/mask, fed per-core as SPMD data.

Dispatch-path note: per-call overhead through the axon-tunneled PJRT
path is ~1ms per operand buffer, so all per-core inputs are packed into
ONE flat f32 DRAM tensor ("blob"); the mask gather table must sit at
blob offset 0 (indirect DMA requires a zero-offset source AP). The
partition-id input is disabled (unused) to drop one more operand.

Blob layout (f32 elements):
  [0        : 131072)  mask_g  (8192 x 16)  gather table, offset 0
  [131072   : 163840)  mask_q  (2048 x 16)  this core's query mask rows
  [163840   : 188416)  pc_all  (8192 x 3)   candidate coords
  [188416   : 194560)  pc_q    (2048 x 3)   this core's query coords

Per core pipeline (per 128-query tile):
  PE:  16 matmuls [4,128]x[4,512] -> PSUM s-chunks
  ACT: copy PSUM -> SBUF row [128, 8192]
  DVE: max8 + find_index8 -> top-8 values/indices; radius filter; index fixup
  SWDGE: indirect gather of mask rows [128, 7x16] (j=0 is self: |diff|=0,
         so it is neither gathered nor summed)
  DVE+ACT: |diff| + accumulate -> per-query partial loss
"""

import numpy as np

import concourse.bass as bass
import concourse.mybir as mybir
import concourse.tile as tile
from concourse import bacc
from concourse.bass import IndirectOffsetOnAxis, ts
from concourse.bass_utils import run_bass_kernel_spmd

B = 2
N = 8192
KS = 16
KNN = 8
R2 = np.float32(0.1) * np.float32(0.1)  # 0.01 squared radius
NCORES = 8
QPC = B * N // NCORES  # 2048 queries per core
NT = QPC // 128        # 16 query tiles per core
CH = 512               # candidate chunk (one PSUM bank)
NCH = N // CH          # 16 chunks

F32 = mybir.dt.float32
U32 = mybir.dt.uint32

# blob element offsets
OFF_MASK_G = 0
OFF_MASK_Q = OFF_MASK_G + N * KS
OFF_PC_ALL = OFF_MASK_Q + QPC * KS
OFF_PC_Q = OFF_PC_ALL + N * 3
BLOB_LEN = OFF_PC_Q + QPC * 3

_CACHE = {}


def _body(tc, blob, loss_out, repeats=1):
    nc = tc.nc
    bap = blob.ap()
    mask_g = bap[OFF_MASK_G : OFF_MASK_G + N * KS].rearrange("(n s) -> n s", s=KS)
    mask_q = bap[OFF_MASK_Q : OFF_MASK_Q + QPC * KS].rearrange("(n s) -> n s", s=KS)
    pc_all = bap[OFF_PC_ALL : OFF_PC_ALL + N * 3].rearrange("(n d) -> d n", d=3)
    pc_q = bap[OFF_PC_Q : OFF_PC_Q + QPC * 3].rearrange("(n d) -> d n", d=3)

    import contextlib
    with contextlib.ExitStack() as ctx:
        cpool = ctx.enter_context(tc.tile_pool(name="const", bufs=1))
        rpool = ctx.enter_context(tc.tile_pool(name="rows", bufs=2))
        spool = ctx.enter_context(tc.tile_pool(name="small", bufs=3))
        ppool = ctx.enter_context(tc.tile_pool(name="psum", bufs=6, space="PSUM"))

        # ---- setup: candidate matrix Cp = [x; y; z; -|c|^2], query matrix
        # Qs = [2x; 2y; 2z; 1] so that s = Qs[:,q].T @ Cp[:,c] = 2 q.c - |c|^2
        Cp = cpool.tile([4, N], F32)
        Qs = cpool.tile([4, QPC], F32)
        # memset the whole tile to 1.0 so row 3 (the "ones" row) is ready,
        # then overwrite rows 0-2 with the coords (DVE/ACT can't start at
        # partition 3, so row 3 is never touched directly by compute).
        nc.vector.memset(Qs[0:4, :], 1.0)
        nc.sync.dma_start(out=Qs[0:3, :], in_=pc_q)
        nc.scalar.mul(Qs[0:3, :], Qs[0:3, :], 2.0)
        nc.sync.dma_start(out=Cp[0:3, :], in_=pc_all)

        sq3 = cpool.tile([3, N], F32)
        nc.vector.tensor_mul(sq3[:, :], Cp[0:3, :], Cp[0:3, :])
        nones3 = cpool.tile([3, 1], F32)
        nc.vector.memset(nones3[:, :], -1.0)
        csqrow = cpool.tile([1, N], F32)
        for ch in range(NCH):
            pcsq = ppool.tile([128, CH], F32, tag="ps")
            nc.tensor.matmul(
                out=pcsq[0:1, :],
                lhsT=nones3[:, :],
                rhs=sq3[:, ts(ch, CH)],
                start=True,
                stop=True,
            )
            nc.scalar.copy(csqrow[0:1, ts(ch, CH)], pcsq[0:1, :])
        # row 3 of Cp = -|c|^2 (DMA has no partition-start restriction)
        nc.sync.dma_start(out=Cp[3:4, :], in_=csqrow[0:1, :])

        # ---- main loop over query tiles
        for _rep in range(repeats):
          for t in range(NT):
            nrow = rpool.tile([128, N], F32)
            for ch in range(NCH):
                ps = ppool.tile([128, CH], F32, tag="ps")
                nc.tensor.matmul(
                    out=ps[:, :],
                    lhsT=Qs[:, ts(t, 128)],
                    rhs=Cp[:, ts(ch, CH)],
                    start=True,
                    stop=True,
                )
                nc.scalar.copy(nrow[:, ts(ch, CH)], ps[:, :])

            # top-8 values (descending) and their indices
            tv = spool.tile([128, 8], F32)
            nc.vector.max(out=tv[:, :], in_=nrow[:, :])
            ti = spool.tile([128, 8], U32)
            nc.vector.max_index(out=ti[:, :], in_max=tv[:, :], in_values=nrow[:, :])

            # keep_j = (s_j >= s_0 - R2)  <=>  d2_j <= R2
            th = spool.tile([128, 1], F32)
            nc.vector.tensor_scalar(
                out=th[:, :], in0=tv[:, 0:1], scalar1=-float(R2), scalar2=None,
                op0=mybir.AluOpType.add,
            )
            kp = spool.tile([128, 8], F32)
            nc.vector.tensor_scalar(
                out=kp[:, :], in0=tv[:, :], scalar1=th[:, :], scalar2=None,
                op0=mybir.AluOpType.is_ge,
            )

            # idx_fixed = idx0 + keep * (idx - idx0)   (all exact in f32)
            idxf = spool.tile([128, 8], F32)
            nc.vector.tensor_copy(idxf[:, :], ti[:, :])
            self_bc = idxf[:, 0:1].to_broadcast([128, 8])
            dl = spool.tile([128, 8], F32)
            nc.vector.tensor_tensor(
                out=dl[:, :], in0=idxf[:, :], in1=self_bc, op=mybir.AluOpType.subtract
            )
            nc.vector.tensor_mul(dl[:, :], dl[:, :], kp[:, :])
            fi = spool.tile([128, 8], F32)
            nc.vector.tensor_tensor(
                out=fi[:, :], in0=dl[:, :], in1=self_bc, op=mybir.AluOpType.add
            )
            fio = spool.tile([128, 8], U32)
            nc.vector.tensor_copy(fio[:, :], fi[:, :])

            # gather neighbor mask rows for j=1..7: [128, 7, 16]
            # (j=0 is always self -> |diff| contribution is exactly 0)
            # ([P,1]-shaped offsets per call: the multi-index offset form
            # compiles but silently transfers nothing on HW)
            gt = spool.tile([128, KNN - 1, KS], F32)
            for j in range(1, KNN):
                nc.gpsimd.indirect_dma_start(
                    out=gt[:, j - 1, :],
                    out_offset=None,
                    in_=mask_g,
                    in_offset=IndirectOffsetOnAxis(ap=fio[:, j : j + 1], axis=0),
                )

            # own mask rows for this tile
            mq = spool.tile([128, KS], F32)
            nc.sync.dma_start(out=mq[:, :], in_=mask_q[ts(t, 128), :])

            # |own - neighbor| summed over (j, s) per query
            df = spool.tile([128, KNN - 1, KS], F32)
            mq_bc = mq[:, :].rearrange("p (o s) -> p o s", o=1).to_broadcast(
                [128, KNN - 1, KS]
            )
            nc.vector.tensor_tensor(
                out=df[:, :, :], in0=gt[:, :, :], in1=mq_bc,
                op=mybir.AluOpType.subtract,
            )
            ab = spool.tile([128, KNN - 1, KS], F32)
            lt = spool.tile([128, 1], F32)
            nc.scalar.activation(
                out=ab[:, :, :], in_=df[:, :, :],
                func=mybir.ActivationFunctionType.Abs,
                accum_out=lt[:, :],
            )
            nc.sync.dma_start(out=loss_out.ap()[:, t : t + 1], in_=lt[:, :])


def build_nc(repeats=1):
    nc = bacc.Bacc(
        "TRN2", target_bir_lowering=False, debug=False, num_devices=NCORES,
        enable_partition_id=False,
    )
    blob = nc.dram_tensor("blob", [BLOB_LEN], F32, kind="ExternalInput")
    loss_out = nc.dram_tensor("loss_out", [128, NT], F32, kind="ExternalOutput")
    with tile.TileContext(nc) as tc:
        _body(tc, blob, loss_out, repeats=repeats)
    nc.compile()
    return nc


def make_in_maps(pc, mask):
    pc = np.ascontiguousarray(np.asarray(pc), dtype=np.float32)
    mask = np.ascontiguousarray(np.asarray(mask), dtype=np.float32)
    in_maps = []
    for c in range(NCORES):
        b, qb = divmod(c, NCORES // B)
        sl = slice(qb * QPC, (qb + 1) * QPC)
        blob = np.concatenate([
            mask[b].reshape(-1),
            mask[b][sl].reshape(-1),
            pc[b].reshape(-1),
            pc[b][sl].reshape(-1),
        ]).astype(np.float32)
        in_maps.append({"blob": blob})
    return in_maps


def kernel(pc, mask):
    if "nc" not in _CACHE:
        _CACHE["nc"] = build_nc()
    nc = _CACHE["nc"]
    res = run_bass_kernel_spmd(nc, make_in_maps(pc, mask), list(range(NCORES)))
    total = 0.0
    for r in res.results:
        total += r["loss_out"].astype(np.float64).sum()
    return np.float32(total / (B * N * KNN))


# revision 3
# speedup vs baseline: 1.1153x; 1.1153x over previous
"""KnnLoss Trainium2 kernel.

Problem: B=2, N=8192 points in [0,1)^3, mask (B,N,16). For each point,
find its 8 nearest neighbors (squared L2, via s = 2*q.c - |c|^2, a
per-row constant shift of -d2), replace out-of-radius neighbors with the
self index, gather mask rows at the neighbor indices, and accumulate
sum_s |mask[n,s] - mask[nn,s]|. Final loss = total / (B*N*k).

Numerics: coords are fp16-quantized (host side); the matmul contraction
is [2x;2y;2z;1;1] . [x;y;z;-H;-L] where H+L is an fp16 hi/lo split of
|c|^2 of the quantized coords, so scores come out of the fp32 PSUM with
d2 error ~1e-4 near the 0.1 radius. The top-8/radius path runs on exact
fp32 scores. Masks are bf16 (|diff| statistics are insensitive). The
j=0 neighbor is always self (|diff| == 0) and is skipped outright.
Measured rel err vs the fp32 reference: 1.4e-05.

Dispatch-path design (axon-tunneled PJRT; per-call overhead dominates):
  - ONE packed u32 input blob (per-operand cost ~1ms/call), mask table
    at offset 0 (indirect DMA needs a zero-offset source AP)
  - partition-id input disabled (one less operand)
  - 2 cores, one batch each (per-call floor scales ~0.33ms/core, and at
    2 cores the query set == candidate set so no extra blob sections)
  - tc.For_i hardware loop over query tiles (per-call cost also scales
    with program size, ~0.5us/instruction)
  - gathers software-pipelined one For_i group behind compute so SWDGE
    emission overlaps the next group's matmul/top-k
"""

import numpy as np

import concourse.bass as bass
import concourse.mybir as mybir
import concourse.tile as tile
from concourse import bacc
from concourse.bass import IndirectOffsetOnAxis, ds, ts
from concourse.bass_utils import run_bass_kernel_spmd

B = 2
N = 8192
KS = 16
KNN = 8
R2 = np.float32(0.1) * np.float32(0.1)

NCORES = 2           # cores used (one per batch)
UNROLL = 8
QPC = B * N // NCORES
NT = QPC // 128

F32 = mybir.dt.float32
F16 = mybir.dt.float16
BF16 = mybir.dt.bfloat16
U32 = mybir.dt.uint32

CH = 512             # matmul chunk (one PSUM bank)
NCH = N // CH
CPY = 2048           # PSUM->SBUF copy span (4 banks)
NCPY = N // CPY

# u32-element blob layout: mask (bf16 pairs) then pc^T (fp16 pairs)
OFF_MASK = 0
OFF_PC = OFF_MASK + N * KS // 2
BLOB_LEN = OFF_PC + N * 3 // 2

_CACHE = {}


def _compute_tile(nc, t, spool, rpool, ppool, Qs3, Cp16, fio3):
    """Matmul + top-8 + radius fixup for tile t; indices -> fio3[:, t, :]."""
    dynamic = not isinstance(t, int)
    tsl = ds(t, 1) if dynamic else slice(t, t + 1)
    if dynamic:
        # walrus can't take a register offset on matmul weights: stage
        # this tile's query block at a fixed SBUF address first.
        qstage = spool.tile([5, 128], F16)
        nc.vector.tensor_copy(qstage[:, :], Qs3[:, tsl, :].squeeze(1))
        lhsT = qstage[:, :]
    else:
        lhsT = Qs3[:, tsl, :].squeeze(1)
    nrow = rpool.tile([128, N], F32)
    for cp in range(NCPY):
        ps = ppool.tile([128, CPY], F32, tag="ps")
        for k in range(CPY // CH):
            ch = cp * (CPY // CH) + k
            nc.tensor.matmul(out=ps[:, ts(k, CH)], lhsT=lhsT,
                             rhs=Cp16[:, ts(ch, CH)], start=True, stop=True)
        nc.scalar.copy(nrow[:, ts(cp, CPY)], ps[:, :])

    tv = spool.tile([128, 8], F32)
    nc.vector.max(out=tv[:, :], in_=nrow[:, :])
    ti = spool.tile([128, 8], U32)
    nc.vector.max_index(out=ti[:, :], in_max=tv[:, :], in_values=nrow[:, :])

    # keep_j = (s_j >= s_0 - R2) <=> d2_j <= R2; fio = where(keep, idx, idx0)
    th = spool.tile([128, 1], F32)
    nc.vector.tensor_scalar(out=th[:, :], in0=tv[:, 0:1], scalar1=-float(R2),
                            scalar2=None, op0=mybir.AluOpType.add)
    kp = spool.tile([128, 8], U32)
    nc.vector.tensor_scalar(out=kp[:, :], in0=tv[:, :], scalar1=th[:, :],
                            scalar2=None, op0=mybir.AluOpType.is_ge)
    fio = fio3[:, tsl, :].squeeze(1)
    nc.vector.tensor_copy(fio, ti[:, 0:1].to_broadcast([128, 8]))
    nc.vector.copy_predicated(fio, kp[:, :], ti[:, :])


def _gather_tile(nc, t, spool, mask_g, mql3, fio3, acc):
    """Gather neighbor mask rows at fio3[:, t, :] and accumulate the loss."""
    dynamic = not isinstance(t, int)
    tsl = ds(t, 1) if dynamic else slice(t, t + 1)
    fio = fio3[:, tsl, :].squeeze(1)
    if dynamic:
        # indirect offset APs must be physical (no register offset)
        fstage = spool.tile([128, 8], U32)
        nc.vector.tensor_copy(fstage[:, :], fio)
        fio = fstage[:, :]
    gt = spool.tile([128, KNN - 1, KS], BF16)
    for j in range(1, KNN):
        nc.gpsimd.indirect_dma_start(
            out=gt[:, j - 1, :], out_offset=None, in_=mask_g,
            in_offset=IndirectOffsetOnAxis(ap=fio[:, j : j + 1], axis=0),
        )
    mq_bc = mql3[:, tsl, :].to_broadcast([128, KNN - 1, KS])
    df = spool.tile([128, KNN - 1, KS], F32)
    nc.vector.tensor_tensor(out=df[:, :, :], in0=gt[:, :, :], in1=mq_bc,
                            op=mybir.AluOpType.subtract)
    ab = spool.tile([128, KNN - 1, KS], F32)
    lt = spool.tile([128, 1], F32)
    nc.scalar.activation(out=ab[:, :, :], in_=df[:, :, :],
                         func=mybir.ActivationFunctionType.Abs,
                         accum_out=lt[:, :])
    nc.vector.tensor_tensor(out=acc[:, :], in0=acc[:, :], in1=lt[:, :],
                            op=mybir.AluOpType.add)


def _body(tc, blob, loss_out, repeats=1):
    nc = tc.nc
    bap = blob.ap()
    mask_g = bap[OFF_MASK : OFF_MASK + N * KS // 2].bitcast(BF16).rearrange(
        "(n s) -> n s", s=KS)
    pc_all = bap[OFF_PC : OFF_PC + N * 3 // 2].bitcast(F16).rearrange(
        "(d n) -> d n", d=3)
    # query mask rows viewed [128, nt, KS]: partition = row % 128
    mask_qT = bap[OFF_MASK : OFF_MASK + QPC * KS // 2].bitcast(BF16).rearrange(
        "(t p s) -> p t s", p=128, s=KS)

    import contextlib
    with contextlib.ExitStack() as ctx:
        cpool = ctx.enter_context(tc.tile_pool(name="const", bufs=1))
        rpool = ctx.enter_context(tc.tile_pool(name="rows", bufs=2))
        spool = ctx.enter_context(tc.tile_pool(name="small", bufs=3))
        ppool = ctx.enter_context(tc.tile_pool(name="psum", bufs=2, space="PSUM"))

        # Cp16 = [x; y; z; -H; -L], Qs16 = [2x; 2y; 2z; 1; 1] (fp16)
        Cp16 = cpool.tile([5, N], F16)
        Qs16 = cpool.tile([5, QPC], F16)
        mql = cpool.tile([128, NT * KS], BF16)
        fioall = cpool.tile([128, NT * 8], U32)
        acc = cpool.tile([128, 1], F32)
        nc.vector.memset(acc[:, :], 0.0)
        nc.vector.memset(Qs16[0:5, :], 1.0)
        nc.sync.dma_start(out=Cp16[0:3, :], in_=pc_all)
        nc.sync.dma_start(
            out=mql.rearrange("p (t s) -> p t s", s=KS), in_=mask_qT)
        nc.scalar.mul(Qs16[0:3, :], Cp16[0:3, :], 2.0)

        with tc.tile_pool(name="setup", bufs=1) as stp:
            sq3 = stp.tile([3, N], F32)
            nc.vector.tensor_mul(sq3[:, :], Cp16[0:3, :], Cp16[0:3, :])
            nones3 = stp.tile([3, 1], F32)
            nc.vector.memset(nones3[:, :], -1.0)
            csq = stp.tile([1, N], F32)
            for ch in range(NCH):
                pcsq = ppool.tile([128, CH], F32, tag="ps")
                nc.tensor.matmul(out=pcsq[0:1, :], lhsT=nones3[:, :],
                                 rhs=sq3[:, ts(ch, CH)], start=True, stop=True)
                nc.scalar.mul(csq[0:1, ts(ch, CH)], pcsq[0:1, :], -1.0)
            negH = stp.tile([1, N], F16)
            nc.vector.tensor_scalar(out=negH[:, :], in0=csq[:, :], scalar1=-1.0,
                                    scalar2=None, op0=mybir.AluOpType.mult)
            nc.vector.tensor_copy(sq3[0:1, :], negH[:, :])
            nc.vector.tensor_tensor(out=csq[:, :], in0=csq[:, :],
                                    in1=sq3[0:1, :], op=mybir.AluOpType.add)
            negL = stp.tile([1, N], F16)
            nc.vector.tensor_scalar(out=negL[:, :], in0=csq[:, :], scalar1=-1.0,
                                    scalar2=None, op0=mybir.AluOpType.mult)
            nc.sync.dma_start(out=Cp16[3:4, :], in_=negH[0:1, :])
            nc.sync.dma_start(out=Cp16[4:5, :], in_=negL[0:1, :])

        Qs3 = Qs16.rearrange("d (t p) -> d t p", p=128)
        mql3 = mql.rearrange("p (t s) -> p t s", s=KS)
        fio3 = fioall.rearrange("p (t e) -> p t e", e=8)

        for _rep in range(repeats):
            for t in range(UNROLL):
                _compute_tile(nc, t, spool, rpool, ppool, Qs3, Cp16, fio3)
            with tc.For_i(0, NT - UNROLL, UNROLL) as t0:
                # gathers first: their DVE stage-copies lead the engine
                # FIFOs so the DMAs launch before this group's compute
                for u in range(UNROLL):
                    _gather_tile(nc, t0 + u, spool, mask_g, mql3, fio3, acc)
                for u in range(UNROLL):
                    _compute_tile(nc, t0 + (u + UNROLL), spool, rpool,
                                  ppool, Qs3, Cp16, fio3)
            for t in range(NT - UNROLL, NT):
                _gather_tile(nc, t, spool, mask_g, mql3, fio3, acc)

        nc.sync.dma_start(out=loss_out.ap()[:, :], in_=acc[:, :])


def build_nc(repeats=1):
    nc = bacc.Bacc("TRN2", target_bir_lowering=False, debug=False,
                   num_devices=NCORES, enable_partition_id=False)
    blob = nc.dram_tensor("blob", [BLOB_LEN], U32, kind="ExternalInput")
    loss_out = nc.dram_tensor("loss_out", [128, 1], F32, kind="ExternalOutput")
    with tile.TileContext(nc) as tc:
        _body(tc, blob, loss_out, repeats=repeats)
    nc.compile()
    return nc


def make_in_maps(pc, mask):
    import ml_dtypes
    pc16 = np.asarray(np.asarray(pc), np.float32).astype(np.float16)
    maskb = np.asarray(np.asarray(mask), np.float32).astype(ml_dtypes.bfloat16)
    in_maps = []
    for b in range(B):
        parts = [
            maskb[b].reshape(-1).view(np.uint32),
            np.ascontiguousarray(pc16[b].T).reshape(-1).view(np.uint32),
        ]
        in_maps.append({"blob": np.concatenate(parts)})
    return in_maps


def kernel(pc, mask):
    if "nc" not in _CACHE:
        _CACHE["nc"] = build_nc()
    nc = _CACHE["nc"]
    res = run_bass_kernel_spmd(nc, make_in_maps(pc, mask), list(range(NCORES)))
    total = 0.0
    for r in res.results:
        total += r["loss_out"].astype(np.float64).sum()
    return np.float32(total / (B * N * KNN))
